# revision 14
# baseline (speedup 1.0000x reference)
"""Trainium2 Bass kernel for nn_EvenOddConvLayer (gnn_message_passing).

Sharding: nodes split across 8 cores (graph parallel). Node tables
(even|odd concatenated) are replicated to every core in DRAM; each core
gathers its neighbors' rows with indirect DMA, so the "all-gather" is
implicit. All dense/elementwise math runs feature-major on-chip; the
only layout transforms are host-side transposes (packing) and one PE
transpose per gathered 128-pair chunk.

The activation engine alternates between two table sets (softplus is
synthesized as ln(exp(x)+1) from natural_log_exp; sigmoid/tanh live in
sigmoid_and_others). ACT work is two-phased per GROUP of tiles and the
ACT instruction order is pinned with explicit dependency edges so the
scheduler cannot interleave the phases (each table switch costs ~2.7us).
The gate/value matmuls run in phase B so sigmoid/tanh read PSUM
directly; their inputs (H, om1 rhs, sv) are staged across the group in
bf16.

kernel(**inputs) takes the FULL unsharded inputs and returns the full
(even_out, odd_out) tuple like the reference.
"""

import numpy as np

import concourse.bacc as bacc
import concourse.mybir as mybir
from concourse.bass import IndirectOffsetOnAxis
from concourse.bass_utils import run_bass_kernel_spmd
from concourse.tile import TileContext, add_dep_helper

# Problem shape (hardcoded per harness contract)
N, M, EVEN, ODD, EDGE = 50000, 20, 64, 64, 32
NCORES = 8
NSHARD = N // NCORES          # 6250 nodes per core
NT = 32                       # nodes per tile
NTILES = 196
NPAD = NT * NTILES            # 6272 padded nodes per core
T = NT * M                    # 640 pairs per tile
NCH = T // 128                # 5 gather chunks per tile
NCHUNKS = NTILES * NCH        # 980
GROUP = 14                    # tiles per ACT table-set phase group
PEI_CH = 448                  # per-node precompute chunk (14 chunks)
F32 = mybir.dt.float32
F16 = mybir.dt.float16
I32 = mybir.dt.int32

AF = mybir.ActivationFunctionType
ALU = mybir.AluOpType

_PROG = None


def _build_program():
    nc = bacc.Bacc("TRN2", target_bir_lowering=False, debug=False)

    # ---- DRAM I/O ----
    nodes_cat = nc.dram_tensor("nodes_cat", [N, 2 * EVEN], F16, kind="ExternalInput")
    idx_cols = nc.dram_tensor("idx_cols", [128, NCHUNKS], I32, kind="ExternalInput")
    nbrT = nc.dram_tensor("nbrT", [EDGE, NPAD * M], F16, kind="ExternalInput")
    evenT_dram = nc.dram_tensor("evenT_ones", [65, NPAD], F32, kind="ExternalInput")
    oddT_dram = nc.dram_tensor("oddT", [64, NPAD], F32, kind="ExternalInput")
    s32_dram = nc.dram_tensor("s32", [128, T], F32, kind="ExternalInput")
    ident_dram = nc.dram_tensor("ident", [128, 128], F16, kind="ExternalInput")
    w_m1_d = nc.dram_tensor("w_m1", [64, 128], F16, kind="ExternalInput")
    w_m2_d = nc.dram_tensor("w_m2", [96, 128], F16, kind="ExternalInput")
    w_cat_i_d = nc.dram_tensor("w_cat_i", [65, 128], F32, kind="ExternalInput")
    w_pei_b_d = nc.dram_tensor("w_pei_b", [65, 64], F32, kind="ExternalInput")
    w_pej_d = nc.dram_tensor("w_pej", [64, 64], F16, kind="ExternalInput")
    # late matmuls run in bf16 (rhs H / om1r are bf16-staged)
    w_eg_d = nc.dram_tensor("w_eg", [64, 64], F16, kind="ExternalInput")
    w_em2_d = nc.dram_tensor("w_em2", [64, 64], F16, kind="ExternalInput")
    # W_og lives at partitions 64-127: its matmul streams H's bottom half
    # into PE rows 64-127, and lhsT must sit on the same partitions
    w_og_d = nc.dram_tensor("w_og", [128, 64], F16, kind="ExternalInput")
    w_om1_d = nc.dram_tensor("w_om1", [128, 64], F16, kind="ExternalInput")
    b_pej_d = nc.dram_tensor("b_pej", [64, 1], F32, kind="ExternalInput")
    b_y_d = nc.dram_tensor("b_y", [128, 1], F32, kind="ExternalInput")
    b_em2_d = nc.dram_tensor("b_em2", [64, 1], F32, kind="ExternalInput")
    outT = nc.dram_tensor("outT", [128, NPAD], F32, kind="ExternalOutput")

    # pin ACT engine order: the per-group two-phase structure only limits
    # table switches if the scheduler cannot reorder ACT instructions
    prev_act = [None]

    def act(out, in_, func, **kw):
        inst = nc.scalar.activation(out, in_, func, **kw)
        if prev_act[0] is not None:
            add_dep_helper(inst.ins, prev_act[0].ins, sync=False,
                           reason="act table-set phase order")
        prev_act[0] = inst
        return inst

    with TileContext(nc) as tc:
        # ---- resident SBUF ----
        with tc.tile_pool(name="resident", bufs=1) as rp:
            idx_sb = rp.tile_from(idx_cols[:, :], name="idx_sb")
            s32 = rp.tile_from(s32_dram[:, :], name="s32_sb")
            ident = rp.tile_from(ident_dram[:, :], name="ident_sb")
            w_m1 = rp.tile_from(w_m1_d[:, :], name="w_m1_sb")
            w_m2 = rp.tile_from(w_m2_d[:, :], name="w_m2_sb")
            w_pej = rp.tile_from(w_pej_d[:, :], name="w_pej_sb")
            w_eg = rp.tile_from(w_eg_d[:, :], name="w_eg_sb")
            w_em2 = rp.tile_from(w_em2_d[:, :], name="w_em2_sb")
            w_og = rp.tile_from(w_og_d[:, :], name="w_og_sb")
            w_om1 = rp.tile_from(w_om1_d[:, :], name="w_om1_sb")
            b_pej = rp.tile_from(b_pej_d[:, :], name="b_pej_sb")
            b_y = rp.tile_from(b_y_d[:, :], name="b_y_sb")
            b_em2 = rp.tile_from(b_em2_d[:, :], name="b_em2_sb")
            # C = [oddT (p0-63) ; pei (p64-127)], feature-major per own node
            C = rp.tile([128, NPAD], F32, tag="C_res")
            # AiGi node-major: node n -> partition n%128, free block n//128
            aigi = rp.tile([128, NPAD], F32, tag="aigi_res")
            outbuf = rp.tile([128, NPAD], F32, tag="outbuf_res")

            nc.sync.dma_start(C[0:64, :], oddT_dram[:, :])

            # ---- per-node precompute: pei (feature-major) and AiGi ----
            with (
                tc.tile_pool(name="pre_sb", bufs=1) as pp,
                tc.tile_pool(name="pre_ps", bufs=2, space="PSUM") as ppp,
            ):
                evenT = pp.tile([65, NPAD], F32, tag="evenT")
                nc.sync.dma_start(evenT[:, :], evenT_dram[:, :])
                w_cat_i = pp.tile_from(w_cat_i_d[:, :], name="w_cat_i_sb")
                w_pei_b = pp.tile_from(w_pei_b_d[:, :], name="w_pei_b_sb")
                # pei[f, n] = (even[n] @ W_pei + b_pei)[f]; bias via ones row,
                # written at partitions 64-127 so downstream bases line up
                for c in range(NPAD // PEI_CH):
                    ps = ppp.tile([128, PEI_CH], F32, tag="pei_ps")
                    sl = slice(c * PEI_CH, (c + 1) * PEI_CH)
                    nc.tensor.matmul(ps[64:128, :], w_pei_b[:, :], evenT[:, sl],
                                     start=True, stop=True,
                                     tile_position=(0, 64))
                    nc.vector.tensor_copy(C[64:128, sl], ps[64:128, :])
                # AiGi feature-major: [A_i|G_i](f, n) + [b_em1|b_ogh] via
                # the evenT ones row; consumed by a DVE broadcast-add
                for c in range(NPAD // PEI_CH):
                    ps = ppp.tile([128, PEI_CH], F32, tag="aigi_ps")
                    sl = slice(c * PEI_CH, (c + 1) * PEI_CH)
                    nc.tensor.matmul(ps[:, :], w_cat_i[:, :], evenT[:, sl],
                                     start=True, stop=True)
                    act(aigi[:, sl], ps[:, :], AF.Copy)

            # ---- main loop pools ----
            with (
                tc.tile_pool(name="gbuf_p", bufs=2) as gp,
                tc.tile_pool(name="gt_p", bufs=2) as gtp,
                tc.tile_pool(name="m2r_p", bufs=2) as m2p,
                tc.tile_pool(name="e_p", bufs=2) as ep,
                tc.tile_pool(name="stage_p", bufs=GROUP + 1) as stp,
                tc.tile_pool(name="sgpr_p", bufs=2) as sgp,
                tc.tile_pool(name="ps_early", bufs=1, space="PSUM") as pse,
                tc.tile_pool(name="ps_p1", bufs=1, space="PSUM") as ps1,
                tc.tile_pool(name="ps_y", bufs=1, space="PSUM") as psy,
                tc.tile_pool(name="ps_z", bufs=1, space="PSUM") as psz,
            ):
                ngroups = (NTILES + GROUP - 1) // GROUP
                for g in range(ngroups):
                    tiles = range(g * GROUP, min((g + 1) * GROUP, NTILES))
                    gt0 = g * GROUP
                    ntl = len(tiles)
                    m2rg = m2p.tile([96, GROUP * T], F16, tag="m2r")
                    nc.sync.dma_start(m2rg[64:96, 0:ntl * T],
                                      nbrT[:, gt0 * T:(gt0 + ntl) * T])
                    staged = {}
                    # ------- phase A (natural_log_exp table set) -------
                    for t in tiles:
                        nsl = slice(t * NT, (t + 1) * NT)        # node cols
                        psl = slice(t * T, (t + 1) * T)          # pair cols
                        gbuf = gp.tile([128, T], F16, tag="gbuf")
                        # one SWDGE op per 128-row chunk: the HW DGE only
                        # supports [P, 1]-shaped offset APs
                        for k in range(NCH):
                            nc.gpsimd.indirect_dma_start(
                                out=gbuf[:, k * 128:(k + 1) * 128],
                                out_offset=None,
                                in_=nodes_cat[:, :],
                                in_offset=IndirectOffsetOnAxis(
                                    ap=idx_sb[:, t * NCH + k:t * NCH + k + 1],
                                    axis=0),
                            )

                        ttr = pse.tile([128, T], F16, tag="early")
                        for k in range(NCH):
                            ksl = slice(k * 128, (k + 1) * 128)
                            nc.tensor.transpose(ttr[:, ksl], gbuf[:, ksl],
                                                ident[:, :])
                        # only even_jT needs to reach SBUF (matmul rhs);
                        # odd_jT is consumed straight from the transpose PSUM
                        gt = gtp.tile([64, T], F16, tag="gt")
                        nc.vector.tensor_copy(gt[:, :], ttr[0:64, :])

                        # cross = odd_i * odd_j -> m2r rows 0-63
                        tl = t - gt0
                        nc.vector.tensor_mul(
                            m2rg[0:64, tl * T:(tl + 1) * T]
                            .rearrange("p (n m) -> p n m", m=M),
                            C[0:64, nsl].to_broadcast([64, NT, M]),
                            ttr[64:128, :].rearrange("p (n m) -> p n m", m=M),
                        )
                        # ei = pei * odd_j; reads odd_jT from the transpose
                        # PSUM, so it must precede the pej tile (shared slot)
                        om1r = stp.tile([128, T], F16, tag="om1r")
                        nc.vector.tensor_mul(
                            om1r[64:128, :].rearrange("p (n m) -> p n m", m=M),
                            ttr[64:128, :].rearrange("p (n m) -> p n m", m=M),
                            C[64:128, nsl].to_broadcast([64, NT, M]))

                        # PSUM1 = W_m1.T@even_j + W_m2.T@[cross;nbr], then
                        # AiGi (incl. biases) added per-node via DVE bcast
                        p1 = ps1.tile([128, 1024], F32, tag="p1")
                        for h in range(2):
                            osl = slice(h * 512, h * 512 + 320)
                            hsl = slice(h * 320, (h + 1) * 320)
                            nc.tensor.matmul(p1[:, osl], w_m1[:, :],
                                             gt[0:64, hsl], start=True, stop=False)
                            nc.tensor.matmul(
                                p1[:, osl], w_m2[:, :],
                                m2rg[:, tl * T + h * 320:tl * T + (h + 1) * 320],
                                start=False, stop=True)
                        p1v = p1[:, :].rearrange("p (b x) -> p b x", x=512)[:, :, 0:320]
                        nc.vector.tensor_add(
                            p1v, p1v,
                            aigi[:, nsl].to_broadcast([128, NT, M]))
                        # softplus(x) = ln(exp(x) + 1): both fns in the
                        # natural_log_exp table set (no native softplus)
                        hh = stp.tile([128, T], F16, tag="hh")
                        e1 = ep.tile([128, T], F32, tag="e1")
                        act(e1[:, :].rearrange("p (b x) -> p b x", x=320),
                            p1v, AF.Exp)
                        act(hh[:, :], e1[:, :], AF.Ln, bias=1.0)

                        # pej -> psum; ie = (pej + b_pej) * odd_i
                        pej = pse.tile([64, 1024], F32, tag="early")
                        for h in range(2):
                            nc.tensor.matmul(pej[:, h * 512:h * 512 + 320],
                                             w_pej[:, :],
                                             gt[:, h * 320:(h + 1) * 320],
                                             start=True, stop=True)
                        pejv = pej[:, :].rearrange("p (b x) -> p b x", x=512)[:, :, 0:320]
                        nc.vector.scalar_tensor_tensor(
                            om1r[0:64, :].rearrange("p (b x) -> p b x", x=320),
                            pejv, b_pej[:, :],
                            C[0:64, nsl].to_broadcast([64, NT, M]),
                            op0=ALU.add, op1=ALU.mult)

                        # em2 head (bf16) into recycled p1-tag PSUM, then
                        # softplus via exp/ln (still phase A table set)
                        zem2 = ps1.tile([64, 1024], F32, tag="p1")
                        for h in range(2):
                            nc.tensor.matmul(zem2[:, h * 512:h * 512 + 320],
                                             w_em2[:, :],
                                             hh[0:64, h * 320:(h + 1) * 320],
                                             start=True, stop=True)
                        zv = zem2[:, :].rearrange("p (b x) -> p b x", x=512)[:, :, 0:320]
                        sv = stp.tile([128, T], F16, tag="sv")
                        e2 = ep.tile([64, T], F32, tag="e2")
                        act(e2[:, :].rearrange("p (b x) -> p b x", x=320),
                            zv, AF.Exp, bias=b_em2[:, :])
                        act(sv[0:64, :], e2[:, :], AF.Ln, bias=1.0)
                        staged[t] = (hh, om1r, sv)

                    # ------- phase B (sigmoid/tanh table set) -------
                    for t in tiles:
                        hh, om1r, sv = staged[t]
                        yps = psy.tile([128, 1024], F32, tag="yps")
                        zps = psz.tile([128, 1024], F32, tag="zps")
                        for h in range(2):
                            osl = slice(h * 512, h * 512 + 320)
                            hsl = slice(h * 320, (h + 1) * 320)
                            nc.tensor.matmul(yps[0:64, osl], w_eg[:, :],
                                             hh[0:64, hsl], start=True,
                                             stop=True, tile_position=(0, 0))
                            nc.tensor.matmul(yps[64:128, osl], w_og[64:128, :],
                                             hh[64:128, hsl], start=True,
                                             stop=True, tile_position=(64, 64))
                            nc.tensor.matmul(zps[64:128, osl], w_om1[:, :],
                                             om1r[:, hsl], start=True,
                                             stop=True, tile_position=(0, 64))
                        ypsv = yps[:, :].rearrange("p (b x) -> p b x", x=512)[:, :, 0:320]
                        zpsv = zps[:, :].rearrange("p (b x) -> p b x", x=512)[:, :, 0:320]
                        sg = sgp.tile([128, T], F16, tag="sg")
                        act(sg[:, :].rearrange("p (b x) -> p b x", x=320),
                            ypsv, AF.Sigmoid, bias=b_y[:, :])
                        act(sv[64:128, :].rearrange("p (b x) -> p b x", x=320),
                            zpsv[64:128], AF.Tanh)
                        pr = sgp.tile([128, T], F16, tag="pr")
                        nc.vector.tensor_mul(pr[:, :], sg[:, :], sv[:, :])
                        nc.vector.reduce_sum(
                            outbuf[:, t * NT:(t + 1) * NT],
                            pr[:, :].rearrange("p (n m) -> p n m", m=M),
                            axis=mybir.AxisListType.X)

                # ---- residual add + store ----
                with tc.tile_pool(name="res_p", bufs=2) as resp:
                    for c in range(NPAD // PEI_CH):
                        sl = slice(c * PEI_CH, (c + 1) * PEI_CH)
                        tmp = resp.tile([128, PEI_CH], F32, tag="res_tmp")
                        nc.sync.dma_start(tmp[0:64, :], evenT_dram[0:64, sl])
                        nc.sync.dma_start(tmp[64:128, :], oddT_dram[:, sl])
                        nc.vector.tensor_add(outbuf[0:64, sl], outbuf[0:64, sl],
                                             tmp[0:64, :])
                        nc.vector.tensor_add(outbuf[64:128, sl],
                                             outbuf[64:128, sl], tmp[64:128, :])
                nc.sync.dma_start(outT[:, :], outbuf[:, :])

    # The act-table-load inserter picks the first table set containing each
    # function: Exp -> exp_and_others, Ln -> natural_log, so every
    # exp-then-ln softplus would reload tables twice (~5us/tile). Hide
    # exp/ln from every set except natural_log_exp_and_others (set indices
    # are positional and unchanged, so runtime table ids stay valid).
    orig_tables = bacc.get_activation_tables

    def _tables_for_softplus(arch):
        tables = orig_tables(arch)
        for name, fns in tables.items():
            if name != "natural_log_exp_and_others":
                fns.discard(AF.Exp)
                fns.discard(AF.Ln)
        return tables

    bacc.get_activation_tables = _tables_for_softplus
    try:
        nc.compile()
    finally:
        bacc.get_activation_tables = orig_tables
    return nc


def _get_program():
    global _PROG
    if _PROG is None:
        _PROG = _build_program()
    return _PROG


def _to_f16(a):
    return np.asarray(a).astype(np.float16)


def _host_prep(even, odd, nbr_fea, idx, W_em1, b_em1, W_eg, b_eg, W_em2, b_em2,
               W_pej, b_pej, W_pei, b_pei, W_om1, W_ogh, b_ogh, W_og, b_og):
    f32 = np.float32
    nodes_cat = np.ascontiguousarray(
        np.concatenate([even, odd], axis=1)).astype(np.float16)  # [N, 128]
    # weights, stacked for the fused heads (cols: [em1 | ogh])
    w_m1 = _to_f16(
        np.concatenate([W_em1[64:128], W_ogh[64:128]], 1))       # even_j rows
    w_m2 = _to_f16(np.concatenate([
        np.concatenate([W_em1[160:224], W_ogh[160:224]], 1),     # cross rows 0-63
        np.concatenate([W_em1[128:160], W_ogh[128:160]], 1),     # nbr rows 64-95
    ], 0))
    w_cat_i = np.ascontiguousarray(np.concatenate([
        np.concatenate([W_em1[0:64], W_ogh[0:64]], 1),
        np.concatenate([b_em1[None, :], b_ogh[None, :]], 1),
    ], 0), f32)                                                  # [65, 128]
    w_pei_b = np.ascontiguousarray(
        np.concatenate([W_pei, b_pei[None, :]], 0), f32)         # [65, 64]
    s32 = (np.arange(T)[None, :] // M == np.arange(128)[:, None] % NT
           ).astype(f32)
    ident = np.eye(128, dtype=np.float16)
    b_y = np.concatenate([b_eg, b_og])[:, None].astype(f32)

    common = dict(
        nodes_cat=nodes_cat, s32=s32, ident=ident,
        w_m1=w_m1, w_m2=w_m2, w_cat_i=w_cat_i, w_pei_b=w_pei_b,
        w_pej=_to_f16(W_pej),
        w_eg=_to_f16(W_eg),
        w_em2=_to_f16(W_em2),
        w_og=_to_f16(np.concatenate([np.zeros((64, 64), f32), W_og], 0)),
        w_om1=_to_f16(W_om1),
        b_pej=np.ascontiguousarray(b_pej[:, None], f32),
        b_y=b_y,
        b_em2=np.ascontiguousarray(b_em2[:, None], f32),
    )

    in_maps = []
    for c in range(NCORES):
        sl = slice(c * NSHARD, (c + 1) * NSHARD)
        ev = np.zeros((NPAD, EVEN), f32); ev[:NSHARD] = even[sl]
        od = np.zeros((NPAD, ODD), f32); od[:NSHARD] = odd[sl]
        nb = np.zeros((NPAD, M, EDGE), np.float16)
        nb[:NSHARD] = nbr_fea[sl]
        ix = np.zeros((NPAD, M), np.int32); ix[:NSHARD] = idx[sl]
        evenT_ones = np.ones((65, NPAD), f32)
        evenT_ones[0:64] = ev.T
        in_maps.append(dict(
            common,
            evenT_ones=np.ascontiguousarray(evenT_ones),
            oddT=np.ascontiguousarray(od.T),
            nbrT=np.ascontiguousarray(nb.reshape(NPAD * M, EDGE).T),
            idx_cols=np.ascontiguousarray(
                ix.reshape(NCHUNKS, 128).T),
        ))
    return in_maps


def kernel(even_node, odd_node, nbr_fea, nbr_fea_idx,
           W_em1, b_em1, W_eg, b_eg, W_em2, b_em2,
           W_pej, b_pej, W_pei, b_pei, W_om1,
           W_ogh, b_ogh, W_og, b_og):
    even = np.asarray(even_node, np.float32)
    odd = np.asarray(odd_node, np.float32)
    nbr = np.asarray(nbr_fea, np.float32)
    idx = np.asarray(nbr_fea_idx).astype(np.int32)
    args = [np.asarray(a, np.float32) for a in (
        W_em1, b_em1, W_eg, b_eg, W_em2, b_em2, W_pej, b_pej,
        W_pei, b_pei, W_om1, W_ogh, b_ogh, W_og, b_og)]
    (W_em1, b_em1, W_eg, b_eg, W_em2, b_em2, W_pej, b_pej,
     W_pei, b_pei, W_om1, W_ogh, b_ogh, W_og, b_og) = args

    nc = _get_program()
    in_maps = _host_prep(even, odd, nbr, idx, W_em1, b_em1, W_eg, b_eg,
                         W_em2, b_em2, W_pej, b_pej, W_pei, b_pei, W_om1,
                         W_ogh, b_ogh, W_og, b_og)
    res = run_bass_kernel_spmd(nc, in_maps, list(range(NCORES)))
    even_out = np.concatenate(
        [r["outT"][0:64, :NSHARD].T for r in res.results], 0)
    odd_out = np.concatenate(
        [r["outT"][64:128, :NSHARD].T for r in res.results], 0)
    return even_out.astype(np.float32), odd_out.astype(np.float32)

